# revision 11
# baseline (speedup 1.0000x reference)
"""Trainium2 Bass kernel for nn_ExtractPatchesPositionLayer.

Reference semantics: per image b, bilinear-translate the (522,522,1) padded
object by t = -positions[b] (tfa.translate: out(y,x) = img(y+py, x+px),
zero fill outside), then center-crop 5px -> (512,512,1).

Because the shift is constant per image, floor/frac of the offset give an
integer window start (A,B) into the (zero-margin-padded) image plus four
constant bilinear corner weights:

    out[r, j] = c00*W[r, j] + c01*W[r, j+1] + c10*W[r+1, j] + c11*W[r+1, j+1]
    W[r, c] = pp[A+r, B+c]

Layout trick: SBUF partition p holds FIVE consecutive padded-image rows
(A+4p .. A+4p+4) as ONE contiguous DRAM span (4*wpad+513 elements, a single
~10.5 KB line-rate DMA descriptor per partition).  Output rows 4p..4p+3 then
depend only on partition p, so the whole bilinear is four fused
multiply-accumulate passes with free-dim shifts (dy*wpad + dx) -- no PE, no
PSUM, no cross-partition shuffles, no remainder chunk.  The output tile holds
4 consecutive y rows per partition = one contiguous 8 KB descriptor each.

DMA routing (hard-won trace facts):
  * inputs: dynamic HWDGE on the SP ring (runtime reg offsets; descriptors
    spread over all 16 SDMA engines by dest SBUF partition).
  * outputs: SWDGE via gpsimd -- HWDGE sends every SBUF->HBM descriptor to
    SDMA engine 0 (1.4 ms serialized); SWDGE's CounterMachine spreads them.
Sharding: batch 256 -> 32 images x 8 cores, embarrassingly parallel.
"""

from dataclasses import dataclass

import numpy as np

import concourse.bacc as bacc
import concourse.bass as bass
import concourse.mybir as mybir
import concourse.tile as tile
from concourse.bass_utils import run_bass_kernel_spmd


@dataclass(frozen=True)
class Cfg:
    bpc: int      # images per core
    n: int        # output height/width
    wpad: int     # padded input height/width (with zero margin)
    xlen: int     # flat padded-input length per core (incl. tail pad)

    @property
    def wrow(self):  # output rows per partition
        return self.n // 128

    @property
    def span(self):  # elements DMA'd per partition (WR+1 full rows + 2)
        return (self.wrow + 1) * self.wpad + 2


def build_nc(cfg: Cfg) -> bass.Bass:
    BPC, N, WPAD = cfg.bpc, cfg.n, cfg.wpad
    WR = cfg.wrow
    SPAN = cfg.span
    WIDE = WR * WPAD  # full-width output row block per partition
    XLEN = cfg.xlen
    f32 = mybir.dt.float32
    i32 = mybir.dt.int32
    MUL = mybir.AluOpType.mult
    ADD = mybir.AluOpType.add

    nc = bacc.Bacc("TRN2", target_bir_lowering=False, debug=False)
    x_d = nc.declare_dram_parameter("x", [1, XLEN], f32, isOutput=False)
    offs_d = nc.declare_dram_parameter("offs", [1, BPC], i32, isOutput=False)
    wmat_d = nc.declare_dram_parameter("wmat", [BPC, 128, 4], f32, isOutput=False)
    y_d = nc.declare_dram_parameter("y", [BPC, N, WPAD], f32, isOutput=True)

    with tile.TileContext(nc) as tc:
        with (
            tc.tile_pool(name="const", bufs=1) as constp,
            tc.tile_pool(name="win", bufs=6) as winp,
            tc.tile_pool(name="hp", bufs=6) as hp,
            tc.tile_pool(name="op", bufs=6) as op,
        ):
            wmat_sb = constp.tile([128, BPC * 4], f32, tag="wmat")
            nc.sync.dma_start(
                wmat_sb[:].rearrange("p (i q) -> p i q", q=4),
                wmat_d[:, :, :].transpose([1, 0, 2]),
            )
            offs_sb = constp.tile([1, BPC], i32, tag="offs")
            nc.sync.dma_start(offs_sb[:], offs_d[:, :])

            regs = [nc.alloc_register(mybir.EngineType.SP, f"dynoff_{k}")
                    for k in range(min(16, BPC))]
            svs = [nc.snap(r, donate=True, min_val=0, max_val=XLEN - 1)
                   for r in regs]
            nreg = len(regs)

            for i in range(BPC):
                k = i % nreg
                nc.sync.reg_load(regs[k], offs_sb[0:1, i: i + 1])
                wt = winp.tile([128, SPAN], f32, tag="wt")
                nc.sync.dma_start(
                    wt[:],
                    bass.AP(x_d, svs[k], [[WR * WPAD, 128], [1, SPAN]]),
                )
                # full-width CONTIGUOUS views (junk between rows is computed
                # and later trimmed on host): DVE runs 2 elem/cycle on flat
                # APs vs 1 on strided 3D ones
                w00 = wt[:, 0: WIDE]
                w01 = wt[:, 1: WIDE + 1]
                w10 = wt[:, WPAD: WPAD + WIDE]
                w11 = wt[:, WPAD + 1: WPAD + WIDE + 1]

                c00 = wmat_sb[:, 4 * i + 0: 4 * i + 1]
                c10 = wmat_sb[:, 4 * i + 1: 4 * i + 2]
                c01 = wmat_sb[:, 4 * i + 2: 4 * i + 3]
                c11 = wmat_sb[:, 4 * i + 3: 4 * i + 4]

                h0 = hp.tile([128, WIDE], f32, tag="h0")
                ob = op.tile([128, WIDE], f32, tag="ob")

                nc.scalar.mul(h0[:], w00, c00)
                nc.vector.scalar_tensor_tensor(h0[:], w01, c01, h0[:], MUL, ADD)
                nc.scalar.mul(ob[:], w10, c10)
                nc.vector.scalar_tensor_tensor(ob[:], w11, c11, ob[:], MUL, ADD)
                nc.gpsimd.tensor_add(ob[:], ob[:], h0[:])

                nc.gpsimd.dma_start(
                    bass.AP(y_d, i * (N * WPAD), [[WIDE, 128], [1, WIDE]]),
                    ob[:],
                )
    nc.compile()
    return nc


def host_prep(padded: np.ndarray, positions: np.ndarray, n_cores: int):
    """Shard + build metadata. padded: (B, npad, npad) f32, positions: (B, 2)."""
    B, npad, _ = padded.shape
    n = npad - 10
    bpc = B // n_cores

    px = positions[:, 0].astype(np.float32)
    py = positions[:, 1].astype(np.float32)
    fy = np.floor(py)
    fx = np.floor(px)
    ay = (5 + fy).astype(np.int64)
    ax = (5 + fx).astype(np.int64)
    wy = (py - fy).astype(np.float32)
    wx = (px - fx).astype(np.float32)

    m_lo = int(max(0, -min(ay.min(), ax.min())))
    m_hi = int(max(0, max(ay.max(), ax.max()) + n + 1 - npad))
    wpad = npad + m_lo + m_hi

    pp = np.zeros((B, wpad, wpad), dtype=np.float32)
    pp[:, m_lo:m_lo + npad, m_lo:m_lo + npad] = padded

    c00 = ((1 - wy) * (1 - wx)).astype(np.float32)
    c10 = (wy * (1 - wx)).astype(np.float32)
    c01 = ((1 - wy) * wx).astype(np.float32)
    c11 = (wy * wx).astype(np.float32)

    A = ay + m_lo
    Bc = ax + m_lo
    base = (np.arange(B, dtype=np.int64) % bpc) * (wpad * wpad)
    off = base + A * wpad + Bc

    wr = n // 128
    span = (wr + 1) * wpad + 2
    # flat length incl. tail so the last image's strided span stays in bounds
    need = int(off.max()) + 127 * wr * wpad + span
    xlen = max(bpc * wpad * wpad, need)

    cfg = Cfg(bpc=bpc, n=n, wpad=wpad, xlen=xlen)

    in_maps = []
    for cidx in range(n_cores):
        sl = slice(cidx * bpc, (cidx + 1) * bpc)
        flat = np.zeros((1, xlen), dtype=np.float32)
        flat[0, :bpc * wpad * wpad] = pp[sl].reshape(-1)
        offs = off[sl].astype(np.int32).reshape(1, bpc)
        wmat = np.empty((bpc, 128, 4), dtype=np.float32)
        wmat[:, :, 0] = c00[sl][:, None]
        wmat[:, :, 1] = c10[sl][:, None]
        wmat[:, :, 2] = c01[sl][:, None]
        wmat[:, :, 3] = c11[sl][:, None]
        in_maps.append({"x": flat, "offs": offs, "wmat": wmat})
    return cfg, in_maps


N_CORES = 8
_nc_cache: dict = {}


def kernel(padded_obj: np.ndarray, positions: np.ndarray) -> np.ndarray:
    padded_obj = np.asarray(padded_obj)
    positions = np.asarray(positions)
    B, npad, _, C = padded_obj.shape
    cfg, in_maps = host_prep(
        padded_obj.reshape(B, npad, npad).astype(np.float32, copy=False),
        positions, N_CORES)

    nc = _nc_cache.get(cfg)
    if nc is None:
        nc = build_nc(cfg)
        _nc_cache[cfg] = nc

    res = run_bass_kernel_spmd(nc, in_maps, core_ids=list(range(N_CORES)))
    out = np.concatenate([r["y"][:, :, :cfg.n] for r in res.results], axis=0)
    return np.ascontiguousarray(out).reshape(B, cfg.n, cfg.n, 1)


# revision 12
# speedup vs baseline: 1.2780x; 1.2780x over previous
"""Trainium2 Bass kernel for nn_ExtractPatchesPositionLayer.

Reference semantics: per image b, bilinear-translate the (522,522,1) padded
object by t = -positions[b] (tfa.translate: out(y,x) = img(y+py, x+px),
zero fill outside), then center-crop 5px -> (512,512,1).

Because the shift is constant per image, floor/frac of the offset give an
integer window start (A,B) into the (zero-margin-padded) image plus four
constant bilinear corner weights:

    out[r, j] = c00*W[r, j] + c01*W[r, j+1] + c10*W[r+1, j] + c11*W[r+1, j+1]
    W[r, c] = pp[A+r, B+c]

Layout trick: SBUF partition p holds FIVE consecutive padded-image rows
(A+4p .. A+4p+4) as ONE contiguous DRAM span (4*wpad+513 elements, a single
~10.5 KB line-rate DMA descriptor per partition).  Output rows 4p..4p+3 then
depend only on partition p, so the whole bilinear is four fused
multiply-accumulate passes with free-dim shifts (dy*wpad + dx) -- no PE, no
PSUM, no cross-partition shuffles, no remainder chunk.  The output tile holds
4 consecutive y rows per partition = one contiguous 8 KB descriptor each.

DMA routing (hard-won trace facts):
  * inputs: dynamic HWDGE on the SP ring (runtime reg offsets; descriptors
    spread over all 16 SDMA engines by dest SBUF partition).
  * outputs: SWDGE via gpsimd -- HWDGE sends every SBUF->HBM descriptor to
    SDMA engine 0 (1.4 ms serialized); SWDGE's CounterMachine spreads them.
Sharding: batch 256 -> 32 images x 8 cores, embarrassingly parallel.
"""

from dataclasses import dataclass

import numpy as np

import concourse.bacc as bacc
import concourse.bass as bass
import concourse.mybir as mybir
import concourse.tile as tile
from concourse.bass_utils import run_bass_kernel_spmd


@dataclass(frozen=True)
class Cfg:
    bpc: int      # images per core
    n: int        # output height/width
    wpad: int     # padded input height/width (with zero margin)
    xlen: int     # flat padded-input length per core (incl. tail pad)

    @property
    def wrow(self):  # output rows per partition
        return self.n // 128

    @property
    def span(self):  # elements DMA'd per partition (WR+1 full rows + 2)
        return (self.wrow + 1) * self.wpad + 2


def build_nc(cfg: Cfg) -> bass.Bass:
    BPC, N, WPAD = cfg.bpc, cfg.n, cfg.wpad
    WR = cfg.wrow
    SPAN = cfg.span
    WIDE = WR * WPAD  # full-width output row block per partition
    XLEN = cfg.xlen
    f32 = mybir.dt.float32
    i32 = mybir.dt.int32
    MUL = mybir.AluOpType.mult
    ADD = mybir.AluOpType.add

    nc = bacc.Bacc("TRN2", target_bir_lowering=False, debug=False)
    x_d = nc.declare_dram_parameter("x", [1, XLEN], f32, isOutput=False)
    offs_d = nc.declare_dram_parameter("offs", [1, BPC], i32, isOutput=False)
    wmat_d = nc.declare_dram_parameter("wmat", [BPC, 128, 4], f32, isOutput=False)
    y_d = nc.declare_dram_parameter("y", [BPC, N, WPAD], f32, isOutput=True)

    with tile.TileContext(nc) as tc:
        with (
            tc.tile_pool(name="const", bufs=1) as constp,
            tc.tile_pool(name="win", bufs=6) as winp,
            tc.tile_pool(name="hp", bufs=6) as hp,
            tc.tile_pool(name="op", bufs=6) as op,
        ):
            wmat_sb = constp.tile([128, BPC * 4], f32, tag="wmat")
            nc.sync.dma_start(
                wmat_sb[:].rearrange("p (i q) -> p i q", q=4),
                wmat_d[:, :, :].transpose([1, 0, 2]),
            )
            offs_sb = constp.tile([1, BPC], i32, tag="offs")
            nc.sync.dma_start(offs_sb[:], offs_d[:, :])

            regs = [nc.alloc_register(mybir.EngineType.SP, f"dynoff_{k}")
                    for k in range(min(16, BPC))]
            svs = [nc.snap(r, donate=True, min_val=0, max_val=XLEN - 1)
                   for r in regs]
            nreg = len(regs)

            for i in range(BPC):
                k = i % nreg
                nc.sync.reg_load(regs[k], offs_sb[0:1, i: i + 1])
                wt = winp.tile([128, SPAN], f32, tag="wt")
                nc.sync.dma_start(
                    wt[:],
                    bass.AP(x_d, svs[k], [[WR * WPAD, 128], [1, SPAN]]),
                )
                # full-width CONTIGUOUS views (junk between rows is computed
                # and later trimmed on host): DVE runs 2 elem/cycle on flat
                # APs vs 1 on strided 3D ones
                w00 = wt[:, 0: WIDE]
                w01 = wt[:, 1: WIDE + 1]
                w10 = wt[:, WPAD: WPAD + WIDE]
                w11 = wt[:, WPAD + 1: WPAD + WIDE + 1]

                c00 = wmat_sb[:, 4 * i + 0: 4 * i + 1]
                c10 = wmat_sb[:, 4 * i + 1: 4 * i + 2]
                c01 = wmat_sb[:, 4 * i + 2: 4 * i + 3]
                c11 = wmat_sb[:, 4 * i + 3: 4 * i + 4]

                h0 = hp.tile([128, WIDE], f32, tag="h0")
                ob = op.tile([128, WIDE], f32, tag="ob")

                nc.scalar.mul(h0[:], w00, c00)
                nc.vector.scalar_tensor_tensor(h0[:], w01, c01, h0[:], MUL, ADD)
                nc.scalar.mul(ob[:], w10, c10)
                nc.vector.scalar_tensor_tensor(ob[:], w11, c11, ob[:], MUL, ADD)
                nc.vector.tensor_add(ob[:], ob[:], h0[:])

                nc.gpsimd.dma_start(
                    bass.AP(y_d, i * (N * WPAD), [[WIDE, 128], [1, WIDE]]),
                    ob[:],
                )
    nc.compile()
    return nc


def host_prep(padded: np.ndarray, positions: np.ndarray, n_cores: int):
    """Shard + build metadata. padded: (B, npad, npad) f32, positions: (B, 2)."""
    B, npad, _ = padded.shape
    n = npad - 10
    bpc = B // n_cores

    px = positions[:, 0].astype(np.float32)
    py = positions[:, 1].astype(np.float32)
    fy = np.floor(py)
    fx = np.floor(px)
    ay = (5 + fy).astype(np.int64)
    ax = (5 + fx).astype(np.int64)
    wy = (py - fy).astype(np.float32)
    wx = (px - fx).astype(np.float32)

    m_lo = int(max(0, -min(ay.min(), ax.min())))
    m_hi = int(max(0, max(ay.max(), ax.max()) + n + 1 - npad))
    wpad = npad + m_lo + m_hi

    pp = np.zeros((B, wpad, wpad), dtype=np.float32)
    pp[:, m_lo:m_lo + npad, m_lo:m_lo + npad] = padded

    c00 = ((1 - wy) * (1 - wx)).astype(np.float32)
    c10 = (wy * (1 - wx)).astype(np.float32)
    c01 = ((1 - wy) * wx).astype(np.float32)
    c11 = (wy * wx).astype(np.float32)

    A = ay + m_lo
    Bc = ax + m_lo
    base = (np.arange(B, dtype=np.int64) % bpc) * (wpad * wpad)
    off = base + A * wpad + Bc

    wr = n // 128
    span = (wr + 1) * wpad + 2
    # flat length incl. tail so the last image's strided span stays in bounds
    need = int(off.max()) + 127 * wr * wpad + span
    xlen = max(bpc * wpad * wpad, need)

    cfg = Cfg(bpc=bpc, n=n, wpad=wpad, xlen=xlen)

    in_maps = []
    for cidx in range(n_cores):
        sl = slice(cidx * bpc, (cidx + 1) * bpc)
        flat = np.zeros((1, xlen), dtype=np.float32)
        flat[0, :bpc * wpad * wpad] = pp[sl].reshape(-1)
        offs = off[sl].astype(np.int32).reshape(1, bpc)
        wmat = np.empty((bpc, 128, 4), dtype=np.float32)
        wmat[:, :, 0] = c00[sl][:, None]
        wmat[:, :, 1] = c10[sl][:, None]
        wmat[:, :, 2] = c01[sl][:, None]
        wmat[:, :, 3] = c11[sl][:, None]
        in_maps.append({"x": flat, "offs": offs, "wmat": wmat})
    return cfg, in_maps


N_CORES = 8
_nc_cache: dict = {}


def kernel(padded_obj: np.ndarray, positions: np.ndarray) -> np.ndarray:
    padded_obj = np.asarray(padded_obj)
    positions = np.asarray(positions)
    B, npad, _, C = padded_obj.shape
    cfg, in_maps = host_prep(
        padded_obj.reshape(B, npad, npad).astype(np.float32, copy=False),
        positions, N_CORES)

    nc = _nc_cache.get(cfg)
    if nc is None:
        nc = build_nc(cfg)
        _nc_cache[cfg] = nc

    res = run_bass_kernel_spmd(nc, in_maps, core_ids=list(range(N_CORES)))
    out = np.concatenate([r["y"][:, :, :cfg.n] for r in res.results], axis=0)
    return np.ascontiguousarray(out).reshape(B, cfg.n, cfg.n, 1)
